# revision 9
# baseline (speedup 1.0000x reference)
"""Trainium2 Bass kernel for nn_MDCR (multi-dilated conv residual block).

Pipeline per batch image (one NeuronCore per batch element, 8 total):
  stage 1: four depthwise 3x3 dilated convs (rates 1/6/12/18, 128 ch each)
           -> +bias -> BN(eval) -> ReLU
  stage 2: shared 1x1 "mix" over the 4 branch outputs (4->4 per channel)
           -> BN -> ReLU
  stage 3: dense 1x1 conv 512->512 -> BN -> ReLU

v4 mapping (engine-balanced per measured HW op costs):
  - Data-parallel over batch: core b handles x[b] fully.
  - Channel-on-partition layout [128, pixels]; branch i owns channels
    128i..128(i+1). All activations bf16 (fp32 accum in PSUM).
  - Branches r=1, r=6 on the PE: 9 taps as diagonal 128x128 matmuls into
    2-bank PSUM tiles per 8-row half-strip; ACT BN+ReLU epilogue.
    W-shifts via zero-padded xpad tiles; H-shifts via row-range clips.
  - Branches r=12, r=18 on the DVE: half-frame (48-row) bf16
    scalar_tensor_tensor FMA chains over W-clipped plain x tiles
    (no padding; the out-of-range tap columns are clipped like rows,
    which also keeps every access 4B-aligned for DVE perf modes).
    Epilogue (scale+bias+relu) on ACT from the SBUF accumulator.
  - Mix + stage 3 entirely on the PE (scaled-identity / dense 128x128
    blocks, FD=512 chunks), ACT epilogues from PSUM.
  - DVE also does the x casts for its two branches; ACT casts the rest.
"""

import ml_dtypes
import numpy as np

import concourse.bass as bass
import concourse.mybir as mybir
import concourse.tile as tile
from concourse.bass_utils import run_bass_kernel_spmd
from concourse.vector_clock import ScopedClock


def _patched_drain_and_barrier(self, tick_clock, wait_clock):
    """This walrus build rejects sync waits on the Drain opcode (CTRL
    NO_STRUCT encoding). Split the kernel-tail drain's aggregated sem waits
    onto individual sync-engine NoOps, then emit a bare drain."""
    nc = self.nc
    collector = nc.sync.nop(nofuse=True, hint="tail_wait_collector")
    wait_clock.add_sem_waits(
        collector.ins, ScopedClock({None: tick_clock.global_clock}))
    si = collector.ins.sync_info
    waits = list(si.on_wait) if si is not None else []
    if len(waits) > 1:
        collector.ins.sync_info = mybir.SyncInfo(
            on_wait=[waits[0]], on_update=list(si.on_update))
        for w in waits[1:]:
            n = nc.sync.nop(nofuse=True, hint="tail_wait")
            n.ins.sync_info = mybir.SyncInfo(on_wait=[w], on_update=[])
    nc.sync.drain()
    nc.all_engine_barrier()
    assert self.sems is not None
    popped = nc._tile_sem_poison_stack.pop()
    assert popped is self._sem_poison
    nc.clear_and_free_semaphores(list(self.sems.allocated().values()))
    nc.all_engine_barrier()


tile.TileContext._drain_and_barrier = _patched_drain_and_barrier


def _split_multi_waits(nc):
    """This walrus build supports at most one sync-wait per instruction.
    Move extra waits onto same-engine NoOps placed immediately before."""
    for fn in nc.m.functions:
        for blk in fn.blocks:
            insts = blk.instructions
            if not any(i.sync_info and len(i.sync_info.on_wait) > 1
                       for i in insts):
                continue
            out = []
            for ins in insts:
                si = ins.sync_info
                if si is not None and len(si.on_wait) > 1:
                    waits = list(si.on_wait)
                    for w in waits[:-1]:
                        nop = mybir.InstNoOp(
                            name=nc.get_next_instruction_name(),
                            sync_info=mybir.SyncInfo(on_wait=[w], on_update=[]),
                            bass_nofuse=True,
                            engine=ins.engine,
                        )
                        try:
                            nc.register_instruction(nop, overwrite=True)
                        except Exception:
                            pass
                        out.append(nop)
                    ins.sync_info = mybir.SyncInfo(
                        on_wait=[waits[-1]], on_update=list(si.on_update))
                out.append(ins)
            blk.instructions = out

EPS = 1e-5
RATES = (1, 6, 12, 18)
B, C, H, W = 8, 512, 96, 96
CQ = C // 4  # 128, one partition chunk per branch
PIX = H * W
BF16 = mybir.dt.bfloat16
F32 = mybir.dt.float32

STRIP_ROWS = 16                 # mix/stage3 strip height
N_STRIPS = H // STRIP_ROWS      # 6
BNT = STRIP_ROWS * W            # 1536 px per strip
BANK_F32 = 512                  # one PSUM bank
CHUNKS = BNT // BANK_F32        # 3
ROWS_PER_BANK = 4               # stage-1 PE psum: 4 rows = 384 px per bank
NT = ROWS_PER_BANK * W
HALF_ROWS = 2 * ROWS_PER_BANK   # 8-row half-strips for stage-1 PE

PE_BRANCHES = (0, 1)            # r=1, r=6 on PE
DVE_BRANCHES = (2, 3)           # r=12, r=18 on DVE
S32_ROWS = 32                   # mix/stage3 mega-strip = one DVE chunk
N_S32 = H // S32_ROWS           # 3
S32_PIX = S32_ROWS * W          # 3072
CPAIRS = S32_PIX // (2 * BANK_F32)  # 3 chunk-pairs per mega-strip

_PROG_CACHE = {}


def _np_bf16(a):
    return np.asarray(a, dtype=np.float32).astype(ml_dtypes.bfloat16)


def _host_consts(wdw, bdw, gdw, bedw, mdw, vdw, ws, bs, gs, bes, ms, vs,
                 wo, bo, go, beo, mo, vo):
    """Fold BN constants and build PE weight blocks on the host."""
    f64 = np.float64
    # stage 1: y = relu(conv * s1 + b1)
    inv1 = np.asarray(gdw, f64) / np.sqrt(np.asarray(vdw, f64) + EPS)  # [4,128]
    s1 = inv1
    b1 = (np.asarray(bdw, f64) - np.asarray(mdw, f64)) * inv1 + np.asarray(bedw, f64)

    # stage 2: z_o = relu(sum_i Amix[o,i]*y_i + b2[o])
    invs = np.asarray(gs, f64) / np.sqrt(np.asarray(vs, f64) + EPS)    # [4]
    W4 = np.asarray(ws, f64)[:, :, 0, 0]                               # [o,i]
    Amix = W4 * invs[:, None]
    b2 = (np.asarray(bs, f64) - np.asarray(ms, f64)) * invs + np.asarray(bes, f64)

    # stage 3: out = relu(Wo' z + b3), Wo' = diag(s3) Wo
    inv3 = np.asarray(go, f64) / np.sqrt(np.asarray(vo, f64) + EPS)    # [512]
    Wo = np.asarray(wo, f64)[:, :, 0, 0]                               # [512,512]
    Wo_s = Wo * inv3[:, None]
    b3 = (np.asarray(bo, f64) - np.asarray(mo, f64)) * inv3 + np.asarray(beo, f64)

    consts = {}
    wdw = np.asarray(wdw, np.float32)
    # PE depthwise diag blocks (branches 0,1): [128, 2*9, 128]
    dw = np.zeros((CQ, 2 * 9, CQ), np.float32)
    for bi, i in enumerate(PE_BRANCHES):
        for t in range(9):
            np.fill_diagonal(dw[:, bi * 9 + t, :], wdw[i, :, 0, t // 3, t % 3])
    consts["dww"] = _np_bf16(dw)

    # mix blocks: [128(k=c), 16(o*4+i), 128(m=c)] = Amix[o,i] * I
    mixw = np.zeros((CQ, 16, CQ), np.float32)
    for o in range(4):
        for i in range(4):
            np.fill_diagonal(mixw[:, o * 4 + i, :], np.float32(Amix[o, i]))
    consts["mixw"] = _np_bf16(mixw)

    # stage-3 blocks: [128(k=c of z-chunk o), 16(m*4+o), 128(mc)]
    # z-chunk o, row c  <->  original z channel 4c+o
    s3w = np.zeros((CQ, 16, CQ), np.float32)
    for m in range(4):
        blk = Wo_s[128 * m:128 * (m + 1), :].astype(np.float32)  # [mc, 512]
        for o in range(4):
            s3w[:, m * 4 + o, :] = blk[:, o::4].T  # [c, mc]
    consts["s3w"] = _np_bf16(s3w)

    # DVE tap weights, raw (scale applied in ACT epilogue): [128, 36]
    dws = np.zeros((CQ, 36), np.float32)
    for i in range(4):
        for t in range(9):
            dws[:, i * 9 + t] = wdw[i, :, 0, t // 3, t % 3]
    consts["dws"] = dws

    consts["s1s"] = np.ascontiguousarray(np.asarray(s1, np.float32).T)  # [128,4]
    consts["s1b"] = np.ascontiguousarray(np.asarray(b1, np.float32).T)  # [128,4]
    consts["b3"] = np.ascontiguousarray(
        np.asarray(b3, np.float32).reshape(4, CQ).T)  # [128,4] col m
    consts["b2"] = np.ascontiguousarray(
        np.broadcast_to(np.asarray(b2, np.float32)[None, :], (CQ, 4))).copy()
    consts["_amix"] = np.asarray(Amix, np.float32)   # host-side immediates
    consts["_b2"] = np.asarray(b2, np.float32)
    return consts


def _tap_list(r, h0, h1):
    """Taps clipped to valid H rows for output rows [h0, h1)."""
    taps = []
    for t in range(9):
        dh, dw = t // 3 - 1, t % 3 - 1
        lo = max(h0, -dh * r)
        hi = min(h1, H - dh * r)
        if lo < hi:
            taps.append((t, dh, dw, lo, hi))
    # center tap first: always covers the full row range
    taps.sort(key=lambda e: (e[3] != h0 or e[4] != h1 or e[1] != 0 or e[2] != 0,))
    return taps


def _build_program(amix, b2, loop_n=None):
    """amix [4,4], b2 [4] are baked in as immediates."""
    nc = bass.Bass("TRN2", target_bir_lowering=False, debug=False, num_devices=8)

    x_d = nc.dram_tensor("x", [C, H, W], F32, kind="ExternalInput")
    dws_d = nc.dram_tensor("dws", [CQ, 36], F32, kind="ExternalInput")
    dww_d = nc.dram_tensor("dww", [CQ, 18, CQ], BF16, kind="ExternalInput")
    mixw_d = nc.dram_tensor("mixw", [CQ, 16, CQ], BF16, kind="ExternalInput")
    s3w_d = nc.dram_tensor("s3w", [CQ, 16, CQ], BF16, kind="ExternalInput")
    s1s_d = nc.dram_tensor("s1s", [CQ, 4], F32, kind="ExternalInput")
    s1b_d = nc.dram_tensor("s1b", [CQ, 4], F32, kind="ExternalInput")
    b3_d = nc.dram_tensor("b3", [CQ, 4], F32, kind="ExternalInput")
    b2_d = nc.dram_tensor("b2", [CQ, 4], F32, kind="ExternalInput")
    out_d = nc.dram_tensor("out", [C, PIX], F32, kind="ExternalOutput")

    with tile.TileContext(nc) as tc:
        with (
            tc.tile_pool(name="consts", bufs=1) as cpool,
            tc.tile_pool(name="xpad", bufs=1) as xpool,
            tc.tile_pool(name="stage", bufs=3) as spool,
            tc.tile_pool(name="yfull", bufs=1) as yfpool,
            tc.tile_pool(name="accs", bufs=4) as accpool,
            tc.tile_pool(name="zs", bufs=4) as zpool,
            tc.tile_pool(name="outs", bufs=3) as opool,
            tc.tile_pool(name="pp", bufs=4, space=bass.MemorySpace.PSUM) as pp,
        ):
          def _body():
            # ---- constants to SBUF
            dww = cpool.tile([CQ, 18, CQ], BF16, tag="dww")
            nc.sync.dma_start(dww[:], dww_d[:])
            mixw = cpool.tile([CQ, 16, CQ], BF16, tag="mixw")
            nc.sync.dma_start(mixw[:], mixw_d[:])
            s3w = cpool.tile([CQ, 16, CQ], BF16, tag="s3w")
            nc.sync.dma_start(s3w[:], s3w_d[:])
            dws = cpool.tile([CQ, 36], F32, tag="dws")
            nc.sync.dma_start(dws[:], dws_d[:])
            s1s = cpool.tile([CQ, 4], F32, tag="s1s")
            nc.sync.dma_start(s1s[:], s1s_d[:])
            s1b = cpool.tile([CQ, 4], F32, tag="s1b")
            nc.sync.dma_start(s1b[:], s1b_d[:])
            b3 = cpool.tile([CQ, 4], F32, tag="b3")
            nc.sync.dma_start(b3[:], b3_d[:])
            b2t = cpool.tile([CQ, 4], F32, tag="b2t")
            nc.sync.dma_start(b2t[:], b2_d[:])

            # ---- x tiles: padded bf16 for PE branches, plain for DVE
            xtile = [None] * 4
            for i in PE_BRANCHES:
                r = RATES[i]
                wp = W + 2 * r
                t = xpool.tile([CQ, H, wp], BF16, tag=f"xpad{i}")
                xtile[i] = t
                nc.gpsimd.memset(t[:, :, 0:r], 0.0)
                nc.gpsimd.memset(t[:, :, r + W:wp], 0.0)
            for i in DVE_BRANCHES:
                xp = xpool.tile([CQ, H, W], BF16, tag=f"xp{i}")
                xtile[i] = xp

            # DVE chunk row ranges: one per 32-row mega-strip
            dve_chunks = [(k * S32_ROWS, (k + 1) * S32_ROWS)
                          for k in range(N_S32)]

            # ---- casts (engine selectable per chunk)
            def cast_dve_chunk(ci, eng):
                c0, c1 = dve_chunks[ci]
                for i in DVE_BRANCHES:
                    for g0 in range(c0, c1, STRIP_ROWS):
                        g1 = g0 + STRIP_ROWS
                        st = spool.tile([CQ, STRIP_ROWS, W], F32, tag="stage")
                        nc.sync.dma_start(
                            st[:], x_d[CQ * i:CQ * (i + 1), g0:g1, :])
                        if eng == "dve":
                            nc.vector.tensor_copy(
                                xtile[i].rearrange("p h w -> p (h w)")
                                [:, g0 * W:g1 * W],
                                st.rearrange("p h w -> p (h w)")[:])
                        else:
                            nc.scalar.activation(
                                xtile[i][:, g0:g1, :], st[:],
                                mybir.ActivationFunctionType.Copy)

            def cast_pe_strip(s):
                g0, g1 = s * STRIP_ROWS, (s + 1) * STRIP_ROWS
                for i in PE_BRANCHES:
                    r = RATES[i]
                    st = spool.tile([CQ, STRIP_ROWS, W], F32, tag="stage")
                    nc.sync.dma_start(
                        st[:], x_d[CQ * i:CQ * (i + 1), g0:g1, :])
                    nc.scalar.activation(
                        xtile[i][:, g0:g1, r:r + W], st[:],
                        mybir.ActivationFunctionType.Copy)

            # ---- y tiles: full-frame for PE branches; DVE branches use
            # their in-place-relu'd acc chunk tiles
            yfull = {}
            for i in PE_BRANCHES:
                yf = yfpool.tile([CQ, PIX], BF16, tag=f"yf{i}")
                yfull[i] = yf

            # in-place ACT relu+BN epilogue makes the acc tile the y chunk
            ydve = {}   # (i, ci) -> y chunk tile

            def dve_stage1_chunk(i, ci, epilogues):
                r = RATES[i]
                c0, c1 = dve_chunks[ci]
                npix = (c1 - c0) * W
                acc = accpool.tile([CQ, S32_PIX], BF16, tag="acc")
                ydve[(i, ci)] = acc
                a3 = acc.rearrange("p (h w) -> p h w", w=W)
                first = True
                for (t, dh, dw, lo, hi) in _tap_list(r, c0, c1):
                    # W-clip: out cols [max(0,-dw*r), min(W, W-dw*r))
                    wlo = max(0, -dw * r)
                    whi = min(W, W - dw * r)
                    xin = xtile[i][:, lo + dh * r:hi + dh * r,
                                   wlo + dw * r:whi + dw * r]
                    sub = a3[:, lo - c0:hi - c0, wlo:whi]
                    sc = dws[:, i * 9 + t:i * 9 + t + 1]
                    if first:
                        first = False
                        assert lo == c0 and hi == c1 and wlo == 0 and whi == W
                        nc.vector.tensor_scalar_mul(
                            acc[:, 0:npix], xtile[i][:, c0:c1, :], sc)
                    else:
                        nc.vector.scalar_tensor_tensor(
                            sub, xin, sc, sub,
                            mybir.AluOpType.mult, mybir.AluOpType.add)
                # epilogue on ACT, in place: y = relu(acc*s1 + b1)
                def _epi(i=i, acc=acc):
                    nc.scalar.activation(
                        acc[:], acc[:],
                        mybir.ActivationFunctionType.Relu,
                        bias=s1b[:, i:i + 1], scale=s1s[:, i:i + 1])
                epilogues.append(_epi)

            # ---- stage 1 on PE (per strip): diag matmuls + ACT epilogue
            def pe_stage1_strip(i, s, ydst):
                r = RATES[i]
                bi = PE_BRANCHES.index(i)
                for half in range(2):
                    h0 = s * STRIP_ROWS + half * HALF_ROWS
                    p1 = pp.tile([CQ, 2, BANK_F32], F32, tag="pp")
                    bidx = [0, 0]
                    btot = [len(_tap_list(r, h0 + b * ROWS_PER_BANK,
                                          h0 + (b + 1) * ROWS_PER_BANK))
                            for b in range(2)]
                    for t in (4, 0, 1, 2, 3, 5, 6, 7, 8):
                        dh, dw = t // 3 - 1, t % 3 - 1
                        for b in range(2):
                            bh0 = h0 + b * ROWS_PER_BANK
                            bh1 = bh0 + ROWS_PER_BANK
                            lo = max(bh0, -dh * r)
                            hi = min(bh1, H - dh * r)
                            if lo >= hi:
                                continue
                            rhs = xtile[i][:, lo + dh * r:hi + dh * r,
                                           r + dw * r:r + dw * r + W]
                            j = bidx[b]
                            bidx[b] += 1
                            nc.tensor.matmul(
                                p1[:, b, (lo - bh0) * W:(hi - bh0) * W],
                                dww[:, bi * 9 + t, :], rhs,
                                start=(j == 0), stop=(j == btot[b] - 1))
                    yh = ydst[:, half * 2 * NT:(half + 1) * 2 * NT]
                    nc.scalar.activation(
                        yh.rearrange("p (b n) -> p b n", b=2),
                        p1[:, :, 0:NT],
                        mybir.ActivationFunctionType.Relu,
                        bias=s1b[:, i:i + 1], scale=s1s[:, i:i + 1])

            # ---- mix + stage 3 for one 32-row mega-strip
            # tiles are [CQ, 2, 512] chunk-pairs from the shared psum ring;
            # epilogues run at FD=1024; LDWEIGHTS amortized over each pair
            def mix_s3_strip(s, ys):
                g0 = s * S32_ROWS
                zs = []
                for o in range(4):
                    z = zpool.tile([CQ, S32_PIX], BF16, tag="z")
                    for cp in range(CPAIRS):
                        p2 = pp.tile([CQ, 2, BANK_F32], F32, tag="pp")
                        for i in range(4):
                            for c in range(2):
                                off = (2 * cp + c) * BANK_F32
                                nc.tensor.matmul(
                                    p2[:, c, :], mixw[:, o * 4 + i, :],
                                    ys[i][:, off:off + BANK_F32],
                                    start=(i == 0), stop=(i == 3))
                        zv = z[:, 2 * cp * BANK_F32:(2 * cp + 2) * BANK_F32]
                        nc.scalar.activation(
                            zv.rearrange("p (b n) -> p b n", b=2), p2[:],
                            mybir.ActivationFunctionType.Relu,
                            bias=b2t[:, o:o + 1], scale=1.0)
                    zs.append(z)

                for m in range(4):
                    for cp in range(CPAIRS):
                        p3 = pp.tile([CQ, 2, BANK_F32], F32, tag="pp")
                        for o in range(4):
                            for c in range(2):
                                off = (2 * cp + c) * BANK_F32
                                nc.tensor.matmul(
                                    p3[:, c, :], s3w[:, m * 4 + o, :],
                                    zs[o][:, off:off + BANK_F32],
                                    start=(o == 0), stop=(o == 3))
                        ot = opool.tile([CQ, 2 * BANK_F32], F32, tag="ot")
                        nc.scalar.activation(
                            ot.rearrange("p (b n) -> p b n", b=2), p3[:],
                            mybir.ActivationFunctionType.Relu,
                            bias=b3[:, m:m + 1], scale=1.0)
                        nc.sync.dma_start(
                            out_d[CQ * m:CQ * (m + 1),
                                  g0 * W + 2 * cp * BANK_F32:
                                  g0 * W + (2 * cp + 2) * BANK_F32], ot[:])

            # ---- emission schedule
            # DVE casts chunk 0 itself (starts its taps ASAP); ACT casts
            # the rest. PE stage-1 epilogues precede DVE's in the ACT
            # stream; each DVE chunk's epilogue is emitted just-in-time
            # before the mega-strip that consumes it.
            cast_dve_chunk(0, "dve")
            cast_pe_strip(0)
            cast_pe_strip(1)
            cast_dve_chunk(1, "act")
            for s in range(2, N_STRIPS):
                cast_pe_strip(s)
            cast_dve_chunk(2, "act")

            for s in range(N_STRIPS):
                for i in PE_BRANCHES:
                    pe_stage1_strip(i, s, yfull[i][:, s * BNT:(s + 1) * BNT])

            def strip_ys(s32):
                ys = [yfull[i][:, s32 * S32_PIX:(s32 + 1) * S32_PIX]
                      for i in PE_BRANCHES]
                ys += [ydve[(i, s32)][:] for i in DVE_BRANCHES]
                return ys

            epis = [[] for _ in range(N_S32)]
            for i in DVE_BRANCHES:
                dve_stage1_chunk(i, 0, epis[0])
            for e in epis[0]:
                e()
            for i in DVE_BRANCHES:
                dve_stage1_chunk(i, 1, epis[1])
            mix_s3_strip(0, strip_ys(0))
            for e in epis[1]:
                e()
            for i in DVE_BRANCHES:
                dve_stage1_chunk(i, 2, epis[2])
            mix_s3_strip(1, strip_ys(1))
            for e in epis[2]:
                e()
            mix_s3_strip(2, strip_ys(2))

          if loop_n:
              with tc.For_i(0, loop_n, 1):
                  _body()
          else:
              _body()
    _split_multi_waits(nc)
    return nc


def _get_program(amix, b2, loop_n=None):
    key = ("v4", loop_n)
    if key not in _PROG_CACHE:
        _PROG_CACHE[key] = _build_program(amix, b2, loop_n)
    return _PROG_CACHE[key]


def _in_maps(x, consts):
    x = np.ascontiguousarray(np.asarray(x, np.float32))
    feed = {k: v for k, v in consts.items() if not k.startswith("_")}
    maps = []
    for b in range(B):
        m = dict(feed)
        m["x"] = np.ascontiguousarray(x[b].reshape(C, H, W))
        maps.append(m)
    return maps


def run(x, consts, trace=False, loop_n=None):
    nc = _get_program(consts["_amix"], consts["_b2"], loop_n)
    res = run_bass_kernel_spmd(nc, _in_maps(x, consts), list(range(B)),
                               trace=trace)
    out = np.stack([res.results[b]["out"].reshape(C, H, W) for b in range(B)])
    return out.astype(np.float32), res


def kernel(x, wdw, bdw, gdw, bedw, mdw, vdw, ws, bs, gs, bes, ms, vs,
           wo, bo, go, beo, mo, vo):
    consts = _host_consts(wdw, bdw, gdw, bedw, mdw, vdw, ws, bs, gs, bes,
                          ms, vs, wo, bo, go, beo, mo, vo)
    out, _ = run(x, consts, trace=False)
    return out


# revision 10
# speedup vs baseline: 1.1326x; 1.1326x over previous
"""Trainium2 Bass kernel for nn_MDCR (multi-dilated conv residual block).

Pipeline per batch image (one NeuronCore per batch element, 8 total):
  stage 1: four depthwise 3x3 dilated convs (rates 1/6/12/18, 128 ch each)
           -> +bias -> BN(eval) -> ReLU
  stage 2: shared 1x1 "mix" over the 4 branch outputs (4->4 per channel)
           -> BN -> ReLU
  stage 3: dense 1x1 conv 512->512 -> BN -> ReLU

v4 mapping (engine-balanced per measured HW op costs):
  - Data-parallel over batch: core b handles x[b] fully.
  - Channel-on-partition layout [128, pixels]; branch i owns channels
    128i..128(i+1). All activations bf16 (fp32 accum in PSUM).
  - Branches r=1, r=6 on the PE: 9 taps as diagonal 128x128 matmuls into
    2-bank PSUM tiles per 8-row half-strip; ACT BN+ReLU epilogue.
    W-shifts via zero-padded xpad tiles; H-shifts via row-range clips.
  - Branches r=12, r=18 on the DVE: half-frame (48-row) bf16
    scalar_tensor_tensor FMA chains over W-clipped plain x tiles
    (no padding; the out-of-range tap columns are clipped like rows,
    which also keeps every access 4B-aligned for DVE perf modes).
    Epilogue (scale+bias+relu) on ACT from the SBUF accumulator.
  - Mix + stage 3 entirely on the PE (scaled-identity / dense 128x128
    blocks, FD=512 chunks), ACT epilogues from PSUM.
  - DVE also does the x casts for its two branches; ACT casts the rest.
"""

import ml_dtypes
import numpy as np

import concourse.bass as bass
import concourse.mybir as mybir
import concourse.tile as tile
from concourse.bass_utils import run_bass_kernel_spmd
from concourse.vector_clock import ScopedClock


def _patched_drain_and_barrier(self, tick_clock, wait_clock):
    """This walrus build rejects sync waits on the Drain opcode (CTRL
    NO_STRUCT encoding). Split the kernel-tail drain's aggregated sem waits
    onto individual sync-engine NoOps, then emit a bare drain."""
    nc = self.nc
    collector = nc.sync.nop(nofuse=True, hint="tail_wait_collector")
    wait_clock.add_sem_waits(
        collector.ins, ScopedClock({None: tick_clock.global_clock}))
    si = collector.ins.sync_info
    waits = list(si.on_wait) if si is not None else []
    if len(waits) > 1:
        collector.ins.sync_info = mybir.SyncInfo(
            on_wait=[waits[0]], on_update=list(si.on_update))
        for w in waits[1:]:
            n = nc.sync.nop(nofuse=True, hint="tail_wait")
            n.ins.sync_info = mybir.SyncInfo(on_wait=[w], on_update=[])
    nc.sync.drain()
    nc.all_engine_barrier()
    assert self.sems is not None
    popped = nc._tile_sem_poison_stack.pop()
    assert popped is self._sem_poison
    nc.clear_and_free_semaphores(list(self.sems.allocated().values()))
    nc.all_engine_barrier()


tile.TileContext._drain_and_barrier = _patched_drain_and_barrier


def _split_multi_waits(nc):
    """This walrus build supports at most one sync-wait per instruction.
    Move extra waits onto same-engine NoOps placed immediately before."""
    for fn in nc.m.functions:
        for blk in fn.blocks:
            insts = blk.instructions
            if not any(i.sync_info and len(i.sync_info.on_wait) > 1
                       for i in insts):
                continue
            out = []
            for ins in insts:
                si = ins.sync_info
                if si is not None and len(si.on_wait) > 1:
                    waits = list(si.on_wait)
                    for w in waits[:-1]:
                        nop = mybir.InstNoOp(
                            name=nc.get_next_instruction_name(),
                            sync_info=mybir.SyncInfo(on_wait=[w], on_update=[]),
                            bass_nofuse=True,
                            engine=ins.engine,
                        )
                        try:
                            nc.register_instruction(nop, overwrite=True)
                        except Exception:
                            pass
                        out.append(nop)
                    ins.sync_info = mybir.SyncInfo(
                        on_wait=[waits[-1]], on_update=list(si.on_update))
                out.append(ins)
            blk.instructions = out

EPS = 1e-5
RATES = (1, 6, 12, 18)
B, C, H, W = 8, 512, 96, 96
CQ = C // 4  # 128, one partition chunk per branch
PIX = H * W
BF16 = mybir.dt.bfloat16
F32 = mybir.dt.float32

STRIP_ROWS = 16                 # mix/stage3 strip height
N_STRIPS = H // STRIP_ROWS      # 6
BNT = STRIP_ROWS * W            # 1536 px per strip
BANK_F32 = 512                  # one PSUM bank
CHUNKS = BNT // BANK_F32        # 3
ROWS_PER_BANK = 4               # stage-1 PE psum: 4 rows = 384 px per bank
NT = ROWS_PER_BANK * W
HALF_ROWS = 2 * ROWS_PER_BANK   # 8-row half-strips for stage-1 PE

PE_BRANCHES = (0, 1)            # r=1, r=6 on PE
DVE_BRANCHES = (2, 3)           # r=12, r=18 on DVE
S32_ROWS = 32                   # mix/stage3 mega-strip = one DVE chunk
N_S32 = H // S32_ROWS           # 3
S32_PIX = S32_ROWS * W          # 3072
CPAIRS = S32_PIX // (2 * BANK_F32)  # 3 chunk-pairs per mega-strip

_PROG_CACHE = {}


def _np_bf16(a):
    return np.asarray(a, dtype=np.float32).astype(ml_dtypes.bfloat16)


def _host_consts(wdw, bdw, gdw, bedw, mdw, vdw, ws, bs, gs, bes, ms, vs,
                 wo, bo, go, beo, mo, vo):
    """Fold BN constants and build PE weight blocks on the host."""
    f64 = np.float64
    # stage 1: y = relu(conv * s1 + b1)
    inv1 = np.asarray(gdw, f64) / np.sqrt(np.asarray(vdw, f64) + EPS)  # [4,128]
    s1 = inv1
    b1 = (np.asarray(bdw, f64) - np.asarray(mdw, f64)) * inv1 + np.asarray(bedw, f64)

    # stage 2: z_o = relu(sum_i Amix[o,i]*y_i + b2[o])
    invs = np.asarray(gs, f64) / np.sqrt(np.asarray(vs, f64) + EPS)    # [4]
    W4 = np.asarray(ws, f64)[:, :, 0, 0]                               # [o,i]
    Amix = W4 * invs[:, None]
    b2 = (np.asarray(bs, f64) - np.asarray(ms, f64)) * invs + np.asarray(bes, f64)

    # stage 3: out = relu(Wo' z + b3), Wo' = diag(s3) Wo
    inv3 = np.asarray(go, f64) / np.sqrt(np.asarray(vo, f64) + EPS)    # [512]
    Wo = np.asarray(wo, f64)[:, :, 0, 0]                               # [512,512]
    Wo_s = Wo * inv3[:, None]
    b3 = (np.asarray(bo, f64) - np.asarray(mo, f64)) * inv3 + np.asarray(beo, f64)

    consts = {}
    wdw = np.asarray(wdw, np.float32)
    # PE depthwise diag blocks (branches 0,1): [128, 2*9, 128]
    dw = np.zeros((CQ, 2 * 9, CQ), np.float32)
    for bi, i in enumerate(PE_BRANCHES):
        for t in range(9):
            np.fill_diagonal(dw[:, bi * 9 + t, :], wdw[i, :, 0, t // 3, t % 3])
    consts["dww"] = _np_bf16(dw)

    # mix blocks: [128(k=c), 16(o*4+i), 128(m=c)] = Amix[o,i] * I
    mixw = np.zeros((CQ, 16, CQ), np.float32)
    for o in range(4):
        for i in range(4):
            np.fill_diagonal(mixw[:, o * 4 + i, :], np.float32(Amix[o, i]))
    consts["mixw"] = _np_bf16(mixw)

    # stage-3 blocks: [128(k=c of z-chunk o), 16(m*4+o), 128(mc)]
    # z-chunk o, row c  <->  original z channel 4c+o
    s3w = np.zeros((CQ, 16, CQ), np.float32)
    for m in range(4):
        blk = Wo_s[128 * m:128 * (m + 1), :].astype(np.float32)  # [mc, 512]
        for o in range(4):
            s3w[:, m * 4 + o, :] = blk[:, o::4].T  # [c, mc]
    consts["s3w"] = _np_bf16(s3w)

    # DVE tap weights, raw (scale applied in ACT epilogue): [128, 36]
    dws = np.zeros((CQ, 36), np.float32)
    for i in range(4):
        for t in range(9):
            dws[:, i * 9 + t] = wdw[i, :, 0, t // 3, t % 3]
    consts["dws"] = dws

    consts["s1s"] = np.ascontiguousarray(np.asarray(s1, np.float32).T)  # [128,4]
    consts["s1b"] = np.ascontiguousarray(np.asarray(b1, np.float32).T)  # [128,4]
    consts["b3"] = np.ascontiguousarray(
        np.asarray(b3, np.float32).reshape(4, CQ).T)  # [128,4] col m
    consts["b2"] = np.ascontiguousarray(
        np.broadcast_to(np.asarray(b2, np.float32)[None, :], (CQ, 4))).copy()
    consts["_amix"] = np.asarray(Amix, np.float32)   # host-side immediates
    consts["_b2"] = np.asarray(b2, np.float32)
    return consts


def _tap_list(r, h0, h1):
    """Taps clipped to valid H rows for output rows [h0, h1)."""
    taps = []
    for t in range(9):
        dh, dw = t // 3 - 1, t % 3 - 1
        lo = max(h0, -dh * r)
        hi = min(h1, H - dh * r)
        if lo < hi:
            taps.append((t, dh, dw, lo, hi))
    # center tap first: always covers the full row range
    taps.sort(key=lambda e: (e[3] != h0 or e[4] != h1 or e[1] != 0 or e[2] != 0,))
    return taps


def _build_program(amix, b2, loop_n=None):
    """amix [4,4], b2 [4] are baked in as immediates."""
    nc = bass.Bass("TRN2", target_bir_lowering=False, debug=False, num_devices=8)

    x_d = nc.dram_tensor("x", [C, H, W], F32, kind="ExternalInput")
    dws_d = nc.dram_tensor("dws", [CQ, 36], F32, kind="ExternalInput")
    dww_d = nc.dram_tensor("dww", [CQ, 18, CQ], BF16, kind="ExternalInput")
    mixw_d = nc.dram_tensor("mixw", [CQ, 16, CQ], BF16, kind="ExternalInput")
    s3w_d = nc.dram_tensor("s3w", [CQ, 16, CQ], BF16, kind="ExternalInput")
    s1s_d = nc.dram_tensor("s1s", [CQ, 4], F32, kind="ExternalInput")
    s1b_d = nc.dram_tensor("s1b", [CQ, 4], F32, kind="ExternalInput")
    b3_d = nc.dram_tensor("b3", [CQ, 4], F32, kind="ExternalInput")
    b2_d = nc.dram_tensor("b2", [CQ, 4], F32, kind="ExternalInput")
    out_d = nc.dram_tensor("out", [C, PIX], BF16, kind="ExternalOutput")

    with tile.TileContext(nc) as tc:
        with (
            tc.tile_pool(name="consts", bufs=1) as cpool,
            tc.tile_pool(name="xpad", bufs=1) as xpool,
            tc.tile_pool(name="stage", bufs=2) as spool,
            tc.tile_pool(name="yfull", bufs=1) as yfpool,
            tc.tile_pool(name="accs", bufs=4) as accpool,
            tc.tile_pool(name="zs", bufs=4) as zpool,
            tc.tile_pool(name="outs", bufs=2) as opool,
            tc.tile_pool(name="pp", bufs=4, space=bass.MemorySpace.PSUM) as pp,
        ):
          def _body():
            # ---- constants to SBUF
            dww = cpool.tile([CQ, 18, CQ], BF16, tag="dww")
            nc.sync.dma_start(dww[:], dww_d[:])
            mixw = cpool.tile([CQ, 16, CQ], BF16, tag="mixw")
            nc.sync.dma_start(mixw[:], mixw_d[:])
            s3w = cpool.tile([CQ, 16, CQ], BF16, tag="s3w")
            nc.sync.dma_start(s3w[:], s3w_d[:])
            dws = cpool.tile([CQ, 36], F32, tag="dws")
            nc.sync.dma_start(dws[:], dws_d[:])
            s1s = cpool.tile([CQ, 4], F32, tag="s1s")
            nc.sync.dma_start(s1s[:], s1s_d[:])
            s1b = cpool.tile([CQ, 4], F32, tag="s1b")
            nc.sync.dma_start(s1b[:], s1b_d[:])
            b3 = cpool.tile([CQ, 4], F32, tag="b3")
            nc.sync.dma_start(b3[:], b3_d[:])
            b2t = cpool.tile([CQ, 4], F32, tag="b2t")
            nc.sync.dma_start(b2t[:], b2_d[:])

            # ---- x tiles: padded bf16 for PE branches, plain for DVE
            xtile = [None] * 4
            for i in PE_BRANCHES:
                r = RATES[i]
                wp = W + 2 * r
                t = xpool.tile([CQ, H, wp], BF16, tag=f"xpad{i}")
                xtile[i] = t
                nc.gpsimd.memset(t[:, :, 0:r], 0.0)
                nc.gpsimd.memset(t[:, :, r + W:wp], 0.0)
            for i in DVE_BRANCHES:
                xp = xpool.tile([CQ, H, W], BF16, tag=f"xp{i}")
                xtile[i] = xp

            # DVE chunk row ranges: one per 32-row mega-strip
            dve_chunks = [(k * S32_ROWS, (k + 1) * S32_ROWS)
                          for k in range(N_S32)]

            # ---- casts: one DMA + one cast per (branch, 32-row chunk)
            def cast_dve_chunk(ci, eng):
                c0, c1 = dve_chunks[ci]
                for i in DVE_BRANCHES:
                    st = spool.tile([CQ, S32_ROWS, W], F32, tag="stage")
                    nc.sync.dma_start(
                        st[:], x_d[CQ * i:CQ * (i + 1), c0:c1, :])
                    if eng == "dve":
                        nc.vector.tensor_copy(
                            xtile[i].rearrange("p h w -> p (h w)")
                            [:, c0 * W:c1 * W],
                            st.rearrange("p h w -> p (h w)")[:])
                    else:
                        nc.scalar.activation(
                            xtile[i][:, c0:c1, :], st[:],
                            mybir.ActivationFunctionType.Copy)

            def cast_pe_chunk(ci):
                c0, c1 = dve_chunks[ci]
                for i in PE_BRANCHES:
                    r = RATES[i]
                    st = spool.tile([CQ, S32_ROWS, W], F32, tag="stage")
                    nc.sync.dma_start(
                        st[:], x_d[CQ * i:CQ * (i + 1), c0:c1, :])
                    nc.scalar.activation(
                        xtile[i][:, c0:c1, r:r + W], st[:],
                        mybir.ActivationFunctionType.Copy)

            # ---- y tiles: full-frame for PE branches; DVE branches use
            # their in-place-relu'd acc chunk tiles
            yfull = {}
            for i in PE_BRANCHES:
                yf = yfpool.tile([CQ, PIX], BF16, tag=f"yf{i}")
                yfull[i] = yf

            # in-place ACT relu+BN epilogue makes the acc tile the y chunk
            ydve = {}   # (i, ci) -> y chunk tile

            def dve_stage1_chunk(i, ci, epilogues):
                r = RATES[i]
                c0, c1 = dve_chunks[ci]
                npix = (c1 - c0) * W
                acc = accpool.tile([CQ, S32_PIX], BF16, tag="acc")
                ydve[(i, ci)] = acc
                a3 = acc.rearrange("p (h w) -> p h w", w=W)
                first = True
                for (t, dh, dw, lo, hi) in _tap_list(r, c0, c1):
                    # W-clip: out cols [max(0,-dw*r), min(W, W-dw*r))
                    wlo = max(0, -dw * r)
                    whi = min(W, W - dw * r)
                    xin = xtile[i][:, lo + dh * r:hi + dh * r,
                                   wlo + dw * r:whi + dw * r]
                    sub = a3[:, lo - c0:hi - c0, wlo:whi]
                    sc = dws[:, i * 9 + t:i * 9 + t + 1]
                    if first:
                        first = False
                        assert lo == c0 and hi == c1 and wlo == 0 and whi == W
                        nc.vector.tensor_scalar_mul(
                            acc[:, 0:npix], xtile[i][:, c0:c1, :], sc)
                    else:
                        nc.vector.scalar_tensor_tensor(
                            sub, xin, sc, sub,
                            mybir.AluOpType.mult, mybir.AluOpType.add)
                # epilogue on ACT, in place: y = relu(acc*s1 + b1)
                def _epi(i=i, acc=acc):
                    nc.scalar.activation(
                        acc[:], acc[:],
                        mybir.ActivationFunctionType.Relu,
                        bias=s1b[:, i:i + 1], scale=s1s[:, i:i + 1])
                epilogues.append(_epi)

            # ---- stage 1 on PE (per strip): diag matmuls + ACT epilogue
            def pe_stage1_strip(i, s, ydst):
                r = RATES[i]
                bi = PE_BRANCHES.index(i)
                for half in range(2):
                    h0 = s * STRIP_ROWS + half * HALF_ROWS
                    p1 = pp.tile([CQ, 2, BANK_F32], F32, tag="pp")
                    bidx = [0, 0]
                    btot = [len(_tap_list(r, h0 + b * ROWS_PER_BANK,
                                          h0 + (b + 1) * ROWS_PER_BANK))
                            for b in range(2)]
                    for t in (4, 0, 1, 2, 3, 5, 6, 7, 8):
                        dh, dw = t // 3 - 1, t % 3 - 1
                        for b in range(2):
                            bh0 = h0 + b * ROWS_PER_BANK
                            bh1 = bh0 + ROWS_PER_BANK
                            lo = max(bh0, -dh * r)
                            hi = min(bh1, H - dh * r)
                            if lo >= hi:
                                continue
                            rhs = xtile[i][:, lo + dh * r:hi + dh * r,
                                           r + dw * r:r + dw * r + W]
                            j = bidx[b]
                            bidx[b] += 1
                            nc.tensor.matmul(
                                p1[:, b, (lo - bh0) * W:(hi - bh0) * W],
                                dww[:, bi * 9 + t, :], rhs,
                                start=(j == 0), stop=(j == btot[b] - 1))
                    yh = ydst[:, half * 2 * NT:(half + 1) * 2 * NT]
                    nc.scalar.activation(
                        yh.rearrange("p (b n) -> p b n", b=2),
                        p1[:, :, 0:NT],
                        mybir.ActivationFunctionType.Relu,
                        bias=s1b[:, i:i + 1], scale=s1s[:, i:i + 1])

            # ---- mix + stage 3 for one 32-row mega-strip
            # tiles are [CQ, 2, 512] chunk-pairs from the shared psum ring;
            # epilogues run at FD=1024; LDWEIGHTS amortized over each pair
            def mix_s3_strip(s, ys):
                g0 = s * S32_ROWS
                zs = []
                for o in range(4):
                    z = zpool.tile([CQ, S32_PIX], BF16, tag="z")
                    for cp in range(CPAIRS):
                        p2 = pp.tile([CQ, 2, BANK_F32], F32, tag="pp")
                        for i in range(4):
                            for c in range(2):
                                off = (2 * cp + c) * BANK_F32
                                nc.tensor.matmul(
                                    p2[:, c, :], mixw[:, o * 4 + i, :],
                                    ys[i][:, off:off + BANK_F32],
                                    start=(i == 0), stop=(i == 3))
                        zv = z[:, 2 * cp * BANK_F32:(2 * cp + 2) * BANK_F32]
                        nc.scalar.activation(
                            zv.rearrange("p (b n) -> p b n", b=2), p2[:],
                            mybir.ActivationFunctionType.Relu,
                            bias=b2t[:, o:o + 1], scale=1.0)
                    zs.append(z)

                for m in range(4):
                    ot = opool.tile([CQ, S32_PIX], BF16, tag="ot")
                    for cp in range(CPAIRS):
                        p3 = pp.tile([CQ, 2, BANK_F32], F32, tag="pp")
                        for o in range(4):
                            for c in range(2):
                                off = (2 * cp + c) * BANK_F32
                                nc.tensor.matmul(
                                    p3[:, c, :], s3w[:, m * 4 + o, :],
                                    zs[o][:, off:off + BANK_F32],
                                    start=(o == 0), stop=(o == 3))
                        ov = ot[:, 2 * cp * BANK_F32:(2 * cp + 2) * BANK_F32]
                        nc.scalar.activation(
                            ov.rearrange("p (b n) -> p b n", b=2), p3[:],
                            mybir.ActivationFunctionType.Relu,
                            bias=b3[:, m:m + 1], scale=1.0)
                    nc.sync.dma_start(
                        out_d[CQ * m:CQ * (m + 1),
                              g0 * W:g0 * W + S32_PIX], ot[:])

            # ---- emission schedule
            # DVE casts chunk 0 itself (starts its taps ASAP); ACT casts
            # the rest. PE stage-1 epilogues precede DVE's in the ACT
            # stream; each DVE chunk's epilogue is emitted just-in-time
            # before the mega-strip that consumes it.
            cast_dve_chunk(0, "dve")
            cast_pe_chunk(0)
            cast_dve_chunk(1, "act")
            cast_pe_chunk(1)
            cast_pe_chunk(2)
            cast_dve_chunk(2, "act")

            for s in range(N_STRIPS):
                for i in PE_BRANCHES:
                    pe_stage1_strip(i, s, yfull[i][:, s * BNT:(s + 1) * BNT])

            def strip_ys(s32):
                ys = [yfull[i][:, s32 * S32_PIX:(s32 + 1) * S32_PIX]
                      for i in PE_BRANCHES]
                ys += [ydve[(i, s32)][:] for i in DVE_BRANCHES]
                return ys

            epis = [[] for _ in range(N_S32)]
            for i in DVE_BRANCHES:
                dve_stage1_chunk(i, 0, epis[0])
            for e in epis[0]:
                e()
            for i in DVE_BRANCHES:
                dve_stage1_chunk(i, 1, epis[1])
            mix_s3_strip(0, strip_ys(0))
            for e in epis[1]:
                e()
            for i in DVE_BRANCHES:
                dve_stage1_chunk(i, 2, epis[2])
            mix_s3_strip(1, strip_ys(1))
            for e in epis[2]:
                e()
            mix_s3_strip(2, strip_ys(2))

          if loop_n:
              with tc.For_i(0, loop_n, 1):
                  _body()
          else:
              _body()
    _split_multi_waits(nc)
    return nc


def _get_program(amix, b2, loop_n=None):
    key = ("v4", loop_n)
    if key not in _PROG_CACHE:
        _PROG_CACHE[key] = _build_program(amix, b2, loop_n)
    return _PROG_CACHE[key]


def _in_maps(x, consts):
    x = np.ascontiguousarray(np.asarray(x, np.float32))
    feed = {k: v for k, v in consts.items() if not k.startswith("_")}
    maps = []
    for b in range(B):
        m = dict(feed)
        m["x"] = np.ascontiguousarray(x[b].reshape(C, H, W))
        maps.append(m)
    return maps


def run(x, consts, trace=False, loop_n=None):
    nc = _get_program(consts["_amix"], consts["_b2"], loop_n)
    res = run_bass_kernel_spmd(nc, _in_maps(x, consts), list(range(B)),
                               trace=trace)
    out = np.stack([res.results[b]["out"].reshape(C, H, W) for b in range(B)])
    return out.astype(np.float32), res


def kernel(x, wdw, bdw, gdw, bedw, mdw, vdw, ws, bs, gs, bes, ms, vs,
           wo, bo, go, beo, mo, vo):
    consts = _host_consts(wdw, bdw, gdw, bedw, mdw, vdw, ws, bs, gs, bes,
                          ms, vs, wo, bo, go, beo, mo, vo)
    out, _ = run(x, consts, trace=False)
    return out
